# revision 7
# baseline (speedup 1.0000x reference)
"""Trainium2 Bass kernel for the LBL BiLM layer (windowed context + Highway).

Computation (per batch b, token t):
  padded   = [left_padding(4), x(1024), right_padding(4)]         # [1032, 512]
  left_ctx[t]  = sum_k lw[k] * padded[t + k]                      # taps t-4..t-1
  right_ctx[t] = sum_k rw[k] * padded[t + 5 + k]                  # taps t+1..t+4
  side_out = Highway_2layer(ctx):  proj = x @ W_i + b_i  (512 -> 1024)
             nl, gate = split(proj); x = sig(gate)*x + (1-sig(gate))*relu(nl)
  out = concat([left_out, right_out], -1)                         # [1024]

Strategy: data-parallel over batch across 8 cores (4 batches each). On-chip
compute happens in *transposed* space (channels on partitions, tokens on the
free axis) so the window taps are free-axis shifted slices (cheap DVE adds)
and the Highway matmuls keep the weights as the stationary operand
(lhsT = W[kc, 128p:..]) with float32r at ~1 cycle/row. PE transposes
(128x128 via identity matmul) convert at the input and output boundaries.
"""

import os
from contextlib import ExitStack

import numpy as np

import concourse.bacc as bacc
import concourse.mybir as mybir
import concourse.tile as tile
from concourse.bass_utils import run_bass_kernel_spmd

AF = mybir.ActivationFunctionType
ALU = mybir.AluOpType
F32 = mybir.dt.float32
F32R = mybir.dt.float32r

N_CORES = 8


def build_program(BL, S, D, W, lw, rw, GS=512):
    """Build the per-core Bass program.

    BL: batches per core; S: seq len; D: model dim; W: window width.
    lw/rw: window weights as python floats (baked into DVE immediates).
    GS: token-group size for the Highway matmuls (moving-operand width).
    """
    nc = bacc.Bacc(
        "TRN2",
        target_bir_lowering=False,
        debug=False,
        enable_asserts=False,
        num_devices=N_CORES,
    )
    D2 = 2 * D
    P = S + 2 * W            # padded rows per batch (1032)
    NDC = D // 128           # d chunks on partitions (4)
    NPC = D2 // 128          # proj chunks (8)
    NG = S // GS             # token groups per batch
    NT = (P + 127) // 128    # natural src tiles per batch (9)
    NTC = GS // 128          # token chunks per group (4)

    x = nc.dram_tensor("x", [BL, S, D], F32, kind="ExternalInput").ap()
    lpad = nc.dram_tensor("lpad", [W, D], F32, kind="ExternalInput").ap()
    rpad = nc.dram_tensor("rpad", [W, D], F32, kind="ExternalInput").ap()
    wl = nc.dram_tensor("wl", [2, D, D2], F32, kind="ExternalInput").ap()
    blv = nc.dram_tensor("blv", [2, D2], F32, kind="ExternalInput").ap()
    wr = nc.dram_tensor("wr", [2, D, D2], F32, kind="ExternalInput").ap()
    brv = nc.dram_tensor("brv", [2, D2], F32, kind="ExternalInput").ap()
    ident = nc.dram_tensor("ident", [128, 128], F32, kind="ExternalInput").ap()
    out = nc.dram_tensor("out", [BL, S, D2], F32, kind="ExternalOutput").ap()

    eq_l = all(v == lw[0] for v in lw)
    eq_r = all(v == rw[0] for v in rw)

    with ExitStack() as ctx:
        tc = ctx.enter_context(tile.TileContext(nc))
        consts = ctx.enter_context(tc.tile_pool(name="consts", bufs=1))
        wpool = ctx.enter_context(tc.tile_pool(name="wpool", bufs=1))
        natp = ctx.enter_context(tc.tile_pool(name="natp", bufs=2))
        ptp = ctx.enter_context(tc.tile_pool(name="ptp", bufs=2))
        ctxp = ctx.enter_context(tc.tile_pool(name="ctxp", bufs=1))
        hwp = ctx.enter_context(tc.tile_pool(name="hwp", bufs=2))
        onatp = ctx.enter_context(tc.tile_pool(name="onatp", bufs=3))
        psin = ctx.enter_context(tc.tile_pool(name="psin", bufs=2, space="PSUM"))
        psmm = ctx.enter_context(tc.tile_pool(name="psmm", bufs=4, space="PSUM"))
        psout = ctx.enter_context(tc.tile_pool(name="psout", bufs=2, space="PSUM"))

        id_sb = consts.tile([128, 128], F32)
        nc.sync.dma_start(id_sb[:], ident[:])

        # Weights resident in SBUF: [side][layer][kc] -> [128, D2]
        wts = {}
        for side, wdram in (("l", wl), ("r", wr)):
            for layer in range(2):
                for kc in range(NDC):
                    wstg = natp.tile([128, D2], F32, name="wstg", tag="nat", bufs=2)
                    nc.sync.dma_start(
                        wstg[:], wdram[layer, 128 * kc:128 * (kc + 1), :]
                    )
                    t = wpool.tile([128, D2], F32R, name=f"w_{side}{layer}{kc}")
                    nc.vector.tensor_copy(t[:], wstg[:])
                    wts[(side, layer, kc)] = t

        # Biases as per-partition columns: [side][layer] -> [128, NPC]
        bias = {}
        for side, bdram in (("l", blv), ("r", brv)):
            for layer in range(2):
                t = consts.tile([128, NPC], F32, name=f"b_{side}{layer}")
                nc.sync.dma_start(t[:], bdram[layer].rearrange("(c p) -> p c", p=128))
                bias[(side, layer)] = t

        for b in range(BL):
            # ---- stage 1: load natural tiles, PE-transpose into PT[dc] ----
            pts = [
                ptp.tile([128, P], F32, name=f"pt{dc}", tag=f"pt{dc}")
                for dc in range(NDC)
            ]
            for i in range(NT):
                r0 = 128 * i
                rows = min(128, P - r0)
                j0, j1 = r0, r0 + rows
                nat = natp.tile([128, D], F32, name="nat", tag="nat", bufs=2)
                # padded row j: j < W -> lpad[j]; j < W+S -> x[b, j-W]; else rpad
                if j0 < W:
                    cnt = min(W, j1) - j0
                    nc.sync.dma_start(nat[0:cnt, :], lpad[j0:j0 + cnt, :])
                mid0, mid1 = max(j0, W), min(j1, W + S)
                if mid1 > mid0:
                    nc.sync.dma_start(
                        nat[mid0 - j0:mid1 - j0, :], x[b, mid0 - W:mid1 - W, :]
                    )
                if j1 > W + S:
                    e0 = max(j0, W + S)
                    nc.sync.dma_start(
                        nat[e0 - j0:rows, :], rpad[e0 - W - S:j1 - W - S, :]
                    )
                ps = psin.tile([128, 512], F32, name="ps_in", tag="ps_in")
                for dc in range(NDC):
                    nc.tensor.transpose(
                        ps[:, 128 * dc:128 * dc + rows],
                        nat[0:rows, 128 * dc:128 * (dc + 1)],
                        id_sb[0:rows, 0:rows],
                    )
                for dc in range(NDC):
                    nc.vector.tensor_copy(
                        pts[dc][:, r0:r0 + rows], ps[:, 128 * dc:128 * dc + rows]
                    )

            # ---- stage 2: windowing on DVE (free-axis shifted slices) ----
            ctxs = {}
            for side, wvals, eq, off in (
                ("l", lw, eq_l, 0),
                ("r", rw, eq_r, W + 1),
            ):
                for dc in range(NDC):
                    pt = pts[dc]
                    c = ctxp.tile(
                        [128, S], F32R, name=f"ctx_{side}{dc}", tag=f"ctx_{side}{dc}"
                    )
                    if eq and W == 4:
                        t1 = ctxp.tile([128, S], F32, name="wtmp", tag="wtmp", bufs=2)
                        nc.vector.tensor_add(
                            t1[:], pt[:, off:off + S], pt[:, off + 1:off + 1 + S]
                        )
                        t2 = ctxp.tile([128, S], F32, name="wtmp", tag="wtmp", bufs=2)
                        nc.vector.tensor_add(
                            t2[:], pt[:, off + 2:off + 2 + S], pt[:, off + 3:off + 3 + S]
                        )
                        nc.vector.tensor_add(c[:], t1[:], t2[:])
                        nc.vector.tensor_scalar_mul(c[:], c[:].bitcast(F32), float(wvals[0]))
                    else:
                        nc.vector.tensor_scalar_mul(
                            c[:], pt[:, off:off + S], float(wvals[0])
                        )
                        for k in range(1, W):
                            tk = ctxp.tile(
                                [128, S], F32, name="wtmp", tag="wtmp", bufs=2
                            )
                            nc.scalar.mul(
                                tk[:], pt[:, off + k:off + k + S], float(wvals[k])
                            )
                            nc.vector.tensor_add(c[:], c[:].bitcast(F32), tk[:])
                    ctxs[(side, dc)] = c

            # ---- stage 3: Highway (2 layers) per token group per side ----
            for g in range(NG):
                onats = [
                    onatp.tile([128, D2], F32, name="onat", tag="onat")
                    for _ in range(NTC)
                ]
                for side in ("l", "r"):
                    inp = [
                        ctxs[(side, dc)][:, GS * g:GS * (g + 1)] for dc in range(NDC)
                    ]
                    for layer in range(2):
                        nlts, gtts = {}, {}
                        for p in range(NPC):
                            pp = psmm.tile([128, GS], F32, name="ps_mm", tag="ps_mm")
                            for kc in range(NDC):
                                nc.tensor.matmul(
                                    pp[:],
                                    wts[(side, layer, kc)][
                                        :, 128 * p:128 * (p + 1)
                                    ],
                                    inp[kc],
                                    start=(kc == 0),
                                    stop=(kc == NDC - 1),
                                )
                            bvec = bias[(side, layer)][:, p:p + 1]
                            if p < NDC:
                                t = hwp.tile([128, GS], F32, name="nl", tag="nl", bufs=6)
                                nc.vector.tensor_scalar(
                                    t[:], pp[:], bvec, 0.0, ALU.add, ALU.max
                                )
                                nlts[p] = t
                            else:
                                t = hwp.tile([128, GS], F32, name="gt", tag="gt", bufs=6)
                                nc.scalar.activation(t[:], pp[:], AF.Sigmoid, bias=bvec)
                                gtts[p - NDC] = t
                        nxt = []
                        for dc in range(NDC):
                            nlt, gtt = nlts[dc], gtts[dc]
                            t1 = hwp.tile([128, GS], F32, name="cmb", tag="cmb", bufs=4)
                            nc.vector.tensor_sub(t1[:], inp[dc].bitcast(F32), nlt[:])
                            nc.vector.tensor_mul(t1[:], t1[:], gtt[:])
                            h = hwp.tile([128, GS], F32R, name="h", tag="h", bufs=8)
                            nc.vector.tensor_add(h[:], t1[:], nlt[:])
                            nxt.append(h[:])
                        inp = nxt

                    # h2 (transposed) -> natural layout via PE transpose
                    side_off = 0 if side == "l" else D
                    for tc2 in range(NTC):
                        po = psout.tile([128, 512], F32, name="ps_out", tag="ps_out")
                        for dc in range(NDC):
                            nc.tensor.transpose(
                                po[:, 128 * dc:128 * (dc + 1)],
                                inp[dc][:, 128 * tc2:128 * (tc2 + 1)].bitcast(F32),
                                id_sb[:],
                            )
                        nc.vector.tensor_copy(
                            onats[tc2][:, side_off:side_off + D], po[:]
                        )
                for tc2 in range(NTC):
                    r0 = g * GS + 128 * tc2
                    nc.sync.dma_start(out[b, r0:r0 + 128, :], onats[tc2][:])

    nc.compile()
    return nc


_CACHE = {}
LAST_RESULTS = None


def _get_program(BL, S, D, W, lw, rw, GS):
    key = (BL, S, D, W, tuple(lw), tuple(rw), GS)
    if key not in _CACHE:
        _CACHE[key] = build_program(BL, S, D, W, list(lw), list(rw), GS)
    return _CACHE[key]


def kernel(inputs, left_padding, right_padding, left_weights, right_weights,
           left_W, left_b, right_W, right_b):
    global LAST_RESULTS
    x = np.ascontiguousarray(np.asarray(inputs, dtype=np.float32))
    B, S, D = x.shape
    lw = [float(v) for v in np.asarray(left_weights)]
    rw = [float(v) for v in np.asarray(right_weights)]
    W = len(lw)
    BL = B // N_CORES
    GS = min(512, S)
    nc = _get_program(BL, S, D, W, lw, rw, GS)

    common = dict(
        lpad=np.ascontiguousarray(np.asarray(left_padding, dtype=np.float32)),
        rpad=np.ascontiguousarray(np.asarray(right_padding, dtype=np.float32)),
        wl=np.ascontiguousarray(np.asarray(left_W, dtype=np.float32)),
        blv=np.ascontiguousarray(np.asarray(left_b, dtype=np.float32)),
        wr=np.ascontiguousarray(np.asarray(right_W, dtype=np.float32)),
        brv=np.ascontiguousarray(np.asarray(right_b, dtype=np.float32)),
        ident=np.eye(128, dtype=np.float32),
    )
    in_maps = [
        dict(x=np.ascontiguousarray(x[c * BL:(c + 1) * BL]), **common)
        for c in range(N_CORES)
    ]
    want_trace = bool(os.environ.get("KERNEL_TRACE"))
    try:
        res = run_bass_kernel_spmd(
            nc, in_maps, list(range(N_CORES)), trace=want_trace,
        )
    except ModuleNotFoundError:
        if not want_trace:
            raise
        res = run_bass_kernel_spmd(nc, in_maps, list(range(N_CORES)), trace=False)
    LAST_RESULTS = res
    last = np.concatenate([res.results[c]["out"] for c in range(N_CORES)], axis=0)
    return (last[None], last)


# revision 8
# speedup vs baseline: 14615.7461x; 14615.7461x over previous
"""Trainium2 Bass kernel for the LBL BiLM layer (windowed context + Highway).

Computation (per batch b, token t):
  padded   = [left_padding(4), x(1024), right_padding(4)]         # [1032, 512]
  left_ctx[t]  = sum_k lw[k] * padded[t + k]                      # taps t-4..t-1
  right_ctx[t] = sum_k rw[k] * padded[t + 5 + k]                  # taps t+1..t+4
  side_out = Highway_2layer(ctx):  proj = x @ W_i + b_i  (512 -> 1024)
             nl, gate = split(proj); x = sig(gate)*x + (1-sig(gate))*relu(nl)
  out = concat([left_out, right_out], -1)                         # [1024]

Strategy: data-parallel over batch across 8 cores (4 batches each). On-chip
compute happens in *transposed* space (channels on partitions, tokens on the
free axis) so the window taps are free-axis shifted slices (cheap DVE adds)
and the Highway matmuls keep the weights as the stationary operand
(lhsT = W[kc, 128p:..]) with float32r at ~1 cycle/row. PE transposes
(128x128 via identity matmul) convert at the input and output boundaries.
"""

import os
from contextlib import ExitStack

import numpy as np

import concourse.bacc as bacc
import concourse.mybir as mybir
import concourse.tile as tile
from concourse.bass_utils import run_bass_kernel_spmd

AF = mybir.ActivationFunctionType
ALU = mybir.AluOpType
F32 = mybir.dt.float32
F32R = mybir.dt.float32r

N_CORES = 8


def build_program(BL, S, D, W, lw, rw, GS=512):
    """Build the per-core Bass program.

    BL: batches per core; S: seq len; D: model dim; W: window width.
    lw/rw: window weights as python floats (baked into DVE immediates).
    GS: token-group size for the Highway matmuls (moving-operand width).
    """
    nc = bacc.Bacc(
        "TRN2",
        target_bir_lowering=False,
        debug=False,
        enable_asserts=False,
        num_devices=N_CORES,
    )
    D2 = 2 * D
    P = S + 2 * W            # padded rows per batch (1032)
    NDC = D // 128           # d chunks on partitions (4)
    NPC = D2 // 128          # proj chunks (8)
    NG = S // GS             # token groups per batch
    NT = (P + 127) // 128    # natural src tiles per batch (9)
    NTC = GS // 128          # token chunks per group (4)

    x = nc.dram_tensor("x", [BL, S, D], F32, kind="ExternalInput").ap()
    lpad = nc.dram_tensor("lpad", [W, D], F32, kind="ExternalInput").ap()
    rpad = nc.dram_tensor("rpad", [W, D], F32, kind="ExternalInput").ap()
    wl = nc.dram_tensor("wl", [2, D, D2], F32, kind="ExternalInput").ap()
    blv = nc.dram_tensor("blv", [2, D2], F32, kind="ExternalInput").ap()
    wr = nc.dram_tensor("wr", [2, D, D2], F32, kind="ExternalInput").ap()
    brv = nc.dram_tensor("brv", [2, D2], F32, kind="ExternalInput").ap()
    ident = nc.dram_tensor("ident", [128, 128], F32, kind="ExternalInput").ap()
    out = nc.dram_tensor("out", [BL, S, D2], F32, kind="ExternalOutput").ap()

    eq_l = all(v == lw[0] for v in lw)
    eq_r = all(v == rw[0] for v in rw)

    with ExitStack() as ctx:
        tc = ctx.enter_context(tile.TileContext(nc))
        consts = ctx.enter_context(tc.tile_pool(name="consts", bufs=1))
        wpool = ctx.enter_context(tc.tile_pool(name="wpool", bufs=1))
        natp = ctx.enter_context(tc.tile_pool(name="natp", bufs=2))
        ptp = ctx.enter_context(tc.tile_pool(name="ptp", bufs=2))
        ctxp = ctx.enter_context(tc.tile_pool(name="ctxp", bufs=1))
        hwp = ctx.enter_context(tc.tile_pool(name="hwp", bufs=2))
        onatp = ctx.enter_context(tc.tile_pool(name="onatp", bufs=3))
        psin = ctx.enter_context(tc.tile_pool(name="psin", bufs=2, space="PSUM"))
        psmm = ctx.enter_context(tc.tile_pool(name="psmm", bufs=4, space="PSUM"))
        psout = ctx.enter_context(tc.tile_pool(name="psout", bufs=2, space="PSUM"))

        id_sb = consts.tile([128, 128], F32)
        nc.sync.dma_start(id_sb[:], ident[:])

        # Weights resident in SBUF: [side][layer][kc] -> [128, D2]
        wts = {}
        for side, wdram in (("l", wl), ("r", wr)):
            for layer in range(2):
                for kc in range(NDC):
                    wstg = natp.tile([128, D2], F32, name="wstg", tag="nat", bufs=2)
                    nc.sync.dma_start(
                        wstg[:], wdram[layer, 128 * kc:128 * (kc + 1), :]
                    )
                    t = wpool.tile([128, D2], F32R, name=f"w_{side}{layer}{kc}")
                    nc.vector.tensor_copy(t[:], wstg[:])
                    wts[(side, layer, kc)] = t

        # Biases as per-partition columns: [side][layer] -> [128, NPC]
        bias = {}
        for side, bdram in (("l", blv), ("r", brv)):
            for layer in range(2):
                t = consts.tile([128, NPC], F32, name=f"b_{side}{layer}")
                nc.sync.dma_start(t[:], bdram[layer].rearrange("(c p) -> p c", p=128))
                bias[(side, layer)] = t

        for b in range(BL):
            # ---- stage 1: load natural tiles, PE-transpose into PT[dc] ----
            pts = [
                ptp.tile([128, P], F32, name=f"pt{dc}", tag=f"pt{dc}")
                for dc in range(NDC)
            ]
            for i in range(NT):
                r0 = 128 * i
                rows = min(128, P - r0)
                j0, j1 = r0, r0 + rows
                nat = natp.tile([128, D], F32, name="nat", tag="nat", bufs=2)
                # padded row j: j < W -> lpad[j]; j < W+S -> x[b, j-W]; else rpad
                if j0 < W:
                    cnt = min(W, j1) - j0
                    nc.sync.dma_start(nat[0:cnt, :], lpad[j0:j0 + cnt, :])
                mid0, mid1 = max(j0, W), min(j1, W + S)
                if mid1 > mid0:
                    nc.sync.dma_start(
                        nat[mid0 - j0:mid1 - j0, :], x[b, mid0 - W:mid1 - W, :]
                    )
                if j1 > W + S:
                    e0 = max(j0, W + S)
                    nc.sync.dma_start(
                        nat[e0 - j0:rows, :], rpad[e0 - W - S:j1 - W - S, :]
                    )
                ps = psin.tile([128, 512], F32, name="ps_in", tag="ps_in")
                for dc in range(NDC):
                    nc.tensor.transpose(
                        ps[:, 128 * dc:128 * dc + rows],
                        nat[0:rows, 128 * dc:128 * (dc + 1)],
                        id_sb[0:rows, 0:rows],
                    )
                for dc in range(NDC):
                    nc.vector.tensor_copy(
                        pts[dc][:, r0:r0 + rows], ps[:, 128 * dc:128 * dc + rows]
                    )

            # ---- stage 2: windowing on DVE (free-axis shifted slices) ----
            ctxs = {}
            for side, wvals, eq, off in (
                ("l", lw, eq_l, 0),
                ("r", rw, eq_r, W + 1),
            ):
                for dc in range(NDC):
                    pt = pts[dc]
                    c = ctxp.tile(
                        [128, S], F32R, name=f"ctx_{side}{dc}", tag=f"ctx_{side}{dc}"
                    )
                    if eq and W == 4:
                        t1 = ctxp.tile([128, S], F32, name="wtmp", tag="wtmp", bufs=2)
                        nc.vector.tensor_add(
                            t1[:], pt[:, off:off + S], pt[:, off + 1:off + 1 + S]
                        )
                        t2 = ctxp.tile([128, S], F32, name="wtmp", tag="wtmp", bufs=2)
                        nc.vector.tensor_add(
                            t2[:], pt[:, off + 2:off + 2 + S], pt[:, off + 3:off + 3 + S]
                        )
                        nc.vector.tensor_add(c[:], t1[:], t2[:])
                        nc.vector.tensor_scalar_mul(c[:], c[:].bitcast(F32), float(wvals[0]))
                    else:
                        nc.vector.tensor_scalar_mul(
                            c[:], pt[:, off:off + S], float(wvals[0])
                        )
                        for k in range(1, W):
                            tk = ctxp.tile(
                                [128, S], F32, name="wtmp", tag="wtmp", bufs=2
                            )
                            nc.scalar.mul(
                                tk[:], pt[:, off + k:off + k + S], float(wvals[k])
                            )
                            nc.vector.tensor_add(c[:], c[:].bitcast(F32), tk[:])
                    ctxs[(side, dc)] = c

            # ---- stage 3: Highway (2 layers) per token group per side ----
            for g in range(NG):
                onats = [
                    onatp.tile([128, D2], F32, name="onat", tag="onat")
                    for _ in range(NTC)
                ]
                for side in ("l", "r"):
                    inp = [
                        ctxs[(side, dc)][:, GS * g:GS * (g + 1)] for dc in range(NDC)
                    ]
                    for layer in range(2):
                        nlts, gtts = {}, {}
                        for p in range(NPC):
                            pp = psmm.tile([128, GS], F32, name="ps_mm", tag="ps_mm")
                            for kc in range(NDC):
                                nc.tensor.matmul(
                                    pp[:],
                                    wts[(side, layer, kc)][
                                        :, 128 * p:128 * (p + 1)
                                    ],
                                    inp[kc],
                                    start=(kc == 0),
                                    stop=(kc == NDC - 1),
                                )
                            bvec = bias[(side, layer)][:, p:p + 1]
                            if p < NDC:
                                t = hwp.tile([128, GS], F32, name="nl", tag="nl", bufs=6)
                                nc.vector.tensor_scalar(
                                    t[:], pp[:], bvec, 0.0, ALU.add, ALU.max
                                )
                                nlts[p] = t
                            else:
                                t = hwp.tile([128, GS], F32, name="gt", tag="gt", bufs=6)
                                nc.scalar.activation(t[:], pp[:], AF.Sigmoid, bias=bvec)
                                gtts[p - NDC] = t
                        nxt = []
                        for dc in range(NDC):
                            nlt, gtt = nlts[dc], gtts[dc]
                            t1 = hwp.tile([128, GS], F32, name="cmb", tag="cmb", bufs=4)
                            nc.vector.tensor_sub(t1[:], inp[dc].bitcast(F32), nlt[:])
                            nc.vector.tensor_mul(t1[:], t1[:], gtt[:])
                            h = hwp.tile([128, GS], F32R, name="h", tag="h", bufs=8)
                            nc.vector.tensor_add(h[:], t1[:], nlt[:])
                            nxt.append(h[:])
                        inp = nxt

                    # h2 (transposed) -> natural layout via PE transpose
                    side_off = 0 if side == "l" else D
                    for tc2 in range(NTC):
                        po = psout.tile([128, 512], F32, name="ps_out", tag="ps_out")
                        for dc in range(NDC):
                            nc.tensor.transpose(
                                po[:, 128 * dc:128 * (dc + 1)],
                                inp[dc][:, 128 * tc2:128 * (tc2 + 1)].bitcast(F32),
                                id_sb[:],
                            )
                        nc.vector.tensor_copy(
                            onats[tc2][:, side_off:side_off + D], po[:]
                        )
                for tc2 in range(NTC):
                    r0 = g * GS + 128 * tc2
                    nc.sync.dma_start(out[b, r0:r0 + 128, :], onats[tc2][:])

    nc.compile()
    return nc


_CACHE = {}
LAST_RESULTS = None


def _get_program(BL, S, D, W, lw, rw, GS):
    key = (BL, S, D, W, tuple(lw), tuple(rw), GS)
    if key not in _CACHE:
        _CACHE[key] = build_program(BL, S, D, W, list(lw), list(rw), GS)
    return _CACHE[key]


def kernel(inputs, left_padding, right_padding, left_weights, right_weights,
           left_W, left_b, right_W, right_b):
    global LAST_RESULTS
    x = np.ascontiguousarray(np.asarray(inputs, dtype=np.float32))
    B, S, D = x.shape
    lw = [float(v) for v in np.asarray(left_weights)]
    rw = [float(v) for v in np.asarray(right_weights)]
    W = len(lw)
    BL = B // N_CORES
    GS = min(512, S)
    nc = _get_program(BL, S, D, W, lw, rw, GS)

    common = dict(
        lpad=np.ascontiguousarray(np.asarray(left_padding, dtype=np.float32)),
        rpad=np.ascontiguousarray(np.asarray(right_padding, dtype=np.float32)),
        wl=np.ascontiguousarray(np.asarray(left_W, dtype=np.float32)),
        blv=np.ascontiguousarray(np.asarray(left_b, dtype=np.float32)),
        wr=np.ascontiguousarray(np.asarray(right_W, dtype=np.float32)),
        brv=np.ascontiguousarray(np.asarray(right_b, dtype=np.float32)),
        ident=np.eye(128, dtype=np.float32),
    )
    in_maps = [
        dict(x=np.ascontiguousarray(x[c * BL:(c + 1) * BL]), **common)
        for c in range(N_CORES)
    ]
    want_trace = bool(os.environ.get("KERNEL_TRACE"))
    tdir = os.environ.get("KERNEL_TRACE_DIR") or None
    if tdir:
        os.makedirs(tdir, exist_ok=True)
    try:
        res = run_bass_kernel_spmd(
            nc, in_maps, list(range(N_CORES)), trace=want_trace, tmpdir=tdir,
        )
    except ModuleNotFoundError:
        if not want_trace:
            raise
        res = run_bass_kernel_spmd(nc, in_maps, list(range(N_CORES)), trace=False)
    LAST_RESULTS = res
    last = np.concatenate([res.results[c]["out"] for c in range(N_CORES)], axis=0)
    return (last[None], last)
